# revision 13
# baseline (speedup 1.0000x reference)
"""Trainium2 Bass kernel: masked multi-head attention, sharded across 8 NeuronCores.

Problem shapes (hardcoded): B=2, T=2048, D=1024, H=16 heads, dh=64.

Sharding: one SPMD program with two phases (one per batch element). In each
phase every core handles 2 of the 16 heads (core c -> heads 2c, 2c+1), so the
16 heads of each batch are spread over all 8 cores. This load-balances the
data-dependent work (Q_len/V_len trim the q/k tile counts per batch).

Device algorithm per phase, per core (all matmul operands fp16; PSUM fp32 —
the fp16 datapath runs the PE at 1 cycle/row vs fp32's 4, and halves DMA):
  - project kT [128=2*64, Kp] and qT [128, Qp] (heads stacked on partition
    halves), and v_aug [128, NK, 2, 65] (token-major with a ones-column at
    index 64 per head, so the PV matmul's psum row 64 is the softmax denom).
    Key masking is done by ZEROING the masked tokens' v_aug rows (so they
    contribute to neither numerator nor denominator) — no exp bias needed,
    which lets several key tiles share one ACT exp instruction.
  - per balanced q chunk (n = Qp/NQC), per group of G=512//n key tiles:
      S^T[kt] = kT_tile.T @ qT_chunk for each kt in group  (one PSUM bank)
      E = exp(scale*S^T)                (ONE ACT instr per group per head)
      [O^T; d] += v_aug.T @ E           (PE, K=128; psum row 64 = denom)
    with a skew-2 software pipeline (S/exp run two groups ahead of PV).
  - epilogue: single DVE copy of the raw [65, n] psum (O^T rows + denom row)
    to fp16 SBUF, DMA out. No on-device normalization.
Host divides O^T rows by the denominator row, applies the query-length mask by
writing only the first qlen rows, and transposes back to [B, T, 1024].
"""

import math
import os
from contextlib import ExitStack

import numpy as np

import concourse.bacc as bacc
import concourse.mybir as mybir
import concourse.tile as tile
from concourse.bass_utils import run_bass_kernel_spmd

F32 = mybir.dt.float32
F16 = mybir.dt.float16
EXP = mybir.ActivationFunctionType.Exp
XDT = F16
XNP = np.float16

B, T, D, H, DH = 2, 2048, 1024, 16, 64
N_CORES = 8
KCH = D // 128          # 8 contraction chunks of the model dim
SCALE = 1.0 / math.sqrt(DH)

LAST_EXEC_NS = None     # filled when BASS_TRACE=1


def _ensure_ntff_hook():
    """run_bass_kernel_spmd(trace=True) imports antenv.axon_hooks, which some
    containers lack; synthesize it (backed by libaxon_pjrt's NRT profiling)
    so tracing degrades gracefully instead of crashing."""
    import sys
    import types
    try:
        import antenv.axon_hooks  # noqa: F401
        return
    except ImportError:
        pass
    try:
        import antenv
        from trn_agent_boot.trn_boot import _ntff_profile_via_ctypes
        hook = _ntff_profile_via_ctypes("/opt/axon/libaxon_pjrt.so")
    except Exception:
        antenv = None
        hook = None
    try:
        m = types.ModuleType("antenv.axon_hooks")
        m._hook = hook
        m.set_axon_ntff_profile_hook = lambda h: setattr(m, "_hook", h)
        m.get_axon_ntff_profile_hook = lambda: m._hook
        sys.modules["antenv.axon_hooks"] = m
        if antenv is not None:
            antenv.axon_hooks = m
    except Exception:
        pass


def _ceil_div(a, b):
    return -(-a // b)


def _chunk_sizes(total, maxn):
    """Split `total` into the fewest chunks of size <= maxn, sizes balanced."""
    nch = _ceil_div(total, maxn)
    base, rem = divmod(total, nch)
    return [base + (1 if i < rem else 0) for i in range(nch)]


def _emit_phase(nc, tc, P, ph):
    """Emit one batch element's phase into the program."""
    s = str(ph["b"])
    io = ph["io"]
    NK, Qp, Kp = ph["NK"], ph["Qp"], ph["Kp"]
    vlen = ph["vlen"]
    scale = ph["scale"]
    wts = P["wts"]

    # --- k/v projections, interleaved per 512-token chunk so the PE gets
    # fresh data as soon as each chunk's DMA lands ---
    kcs = []
    vas = []
    kchunks = _chunk_sizes(Kp, 512)
    for c, n in enumerate(kchunks):
        off = sum(kchunks[:c])
        # k chunk -> kT [128(2h*64d), n]
        xt = P["x"].tile([128, KCH, n], XDT, tag="xt", name="xt")
        if ph.get("first") and c == 0:
            # per-k-slice DMAs let the first projection matmul start as
            # soon as slice 0 lands instead of after the whole chunk
            for k in range(KCH):
                nc.gpsimd.dma_start(xt[:, k, :], io["xk"][:, k, off:off + n])
        else:
            nc.gpsimd.dma_start(xt[:], io["xk"][:, :, off:off + n])
        ps = P["pp"].tile([128, n], F32, tag="pp", name="pp")
        for k in range(KCH):
            nc.tensor.matmul(ps[:], lhsT=wts["wk"][:, k, :], rhs=xt[:, k, :],
                             start=(k == 0), stop=(k == KCH - 1))
        kc = P["persist"].tile([128, n], XDT, tag="kT" + s, name="kT" + s,
                               bufs=len(kchunks))
        nc.vector.tensor_copy(kc[:], ps[:])
        kcs.append((off, n, kc))

        # v chunk -> v_aug tiles [128 tokens, 2 heads, 1+64]
        xtv = P["x"].tile([128, KCH, n], XDT, tag="xt", name="xt")
        nc.gpsimd.dma_start(xtv[:], io["xv"][:, :, off:off + n])
        for m in range(n // 128):
            kt = off // 128 + m
            va = P["persist"].tile([128, 2, 65], XDT, tag="va" + s, name="va" + s,
                                   bufs=NK)
            if vlen > 0 and kt == NK - 1 and vlen - kt * 128 < 128:
                # masked tail keys: zero ones-column rows (the host already
                # zeroed their xv columns, so the v rows are zero) — they
                # then contribute to neither numerator nor denominator,
                # replacing the exp bias mask. Partition slices must start
                # 32-aligned, so zero the whole column first.
                nc.vector.memset(va[:, :, 64:65], 0.0)
                nc.vector.memset(va[0:vlen - kt * 128, :, 64:65], 1.0)
            else:
                nc.vector.memset(va[:, :, 64:65], 1.0)
            ps = P["pp"].tile([128, 128], F32, tag="pp", name="pp")
            for k in range(KCH):
                nc.tensor.matmul(ps[:], lhsT=xtv[:, k, m * 128:(m + 1) * 128],
                                 rhs=wts["wv"][:, k, :],
                                 start=(k == 0), stop=(k == KCH - 1))
            nc.vector.tensor_copy(va[:, :, 0:64],
                                  ps[:].rearrange("p (g d) -> p g d", g=2))
            vas.append(va)

    def kc_slice(kt):
        for off, n, kc in kcs:
            if off <= kt * 128 < off + n:
                return kc, kt * 128 - off
        raise AssertionError

    # --- q projection + attention over balanced q chunks ---
    qchunks = _chunk_sizes(Qp, 512)
    NQC = len(qchunks)

    def emit_qproj(c):
        n = qchunks[c]
        off = sum(qchunks[:c])
        xt = P["x"].tile([128, KCH, n], XDT, tag="xtq", name="xtq", bufs=2)
        nc.gpsimd.dma_start(xt[:], io["xq"][:, :, off:off + n])
        ps = P["pp"].tile([128, n], F32, tag="pp", name="pp")
        for k in range(KCH):
            nc.tensor.matmul(ps[:], lhsT=wts["wq"][:, k, :], rhs=xt[:, k, :],
                             start=(k == 0), stop=(k == KCH - 1))
        qc = P["persist"].tile([128, n], XDT, tag="qT" + s, name="qT" + s,
                               bufs=3)
        nc.vector.tensor_copy(qc[:], ps[:])
        return qc

    qcs = {0: emit_qproj(0)}
    for c in range(NQC):
        n = qchunks[c]
        off = sum(qchunks[:c])
        qc = qcs.pop(c)

        otd = [P["ot"].tile([65, n], F32, tag="otd", name="otd") for _ in (0, 1)]

        # group key tiles so one exp instruction covers G of them (bias-free
        # exp makes this legal; G*n must fit one 512-float PSUM bank)
        gmax = max(1, 512 // n)
        gsizes = _chunk_sizes(NK, gmax)
        gstart = [sum(gsizes[:i]) for i in range(len(gsizes))]

        def emit_s(gi):
            g0, gn = gstart[gi], gsizes[gi]
            es = []
            for h in (0, 1):
                sps = P["sp"].tile([128, gn, n], F32, tag="sps", name="sps")
                for j in range(gn):
                    kc, ko = kc_slice(g0 + j)
                    nc.tensor.matmul(
                        sps[:, j, :],
                        lhsT=kc[h * 64:(h + 1) * 64, ko:ko + 128],
                        rhs=qc[h * 64:(h + 1) * 64, :],
                        start=True, stop=True)
                e = P["e"].tile([128, gn, n], XDT, tag="e", name="e")
                # squeeze to a 2-D access pattern: a leading size-1 dim costs
                # the ACT ~100ns of extra per-instruction overhead
                nc.scalar.activation(
                    e[:].rearrange("p g n -> p (g n)"),
                    sps[:].rearrange("p g n -> p (g n)"), EXP, scale=scale)
                es.append(e)
            return es

        # skew-2 software pipeline: S/exp run two groups ahead of the PV
        # matmuls, so the PE never waits on the ACT exp
        NG = len(gsizes)
        pend = {0: emit_s(0)}
        if NG > 1:
            pend[1] = emit_s(1)
        for gi in range(NG):
            es_cur = pend.pop(gi)
            if gi + 2 < NG:
                pend[gi + 2] = emit_s(gi + 2)
            g0, gn = gstart[gi], gsizes[gi]
            for j in range(gn):
                kt = g0 + j
                for h in (0, 1):
                    nc.tensor.matmul(otd[h][:], lhsT=vas[kt][:, h, :],
                                     rhs=es_cur[h][:, j, :],
                                     start=(kt == 0), stop=(kt == NK - 1),
                                     skip_group_check=True)
        if c + 1 < NQC:
            qcs[c + 1] = emit_qproj(c + 1)
        for h in (0, 1):
            # ship raw numerator rows + denominator row; host normalizes
            osb = P["rows"].tile([65, n], F16, tag="osb", name="osb")
            nc.vector.tensor_copy(osb[:], otd[h][:])
            nc.gpsimd.dma_start(io["out"][h, :, off:off + n], osb[:])


def _build_program(phases):
    nc = bacc.Bacc("TRN2", target_bir_lowering=False, debug=False,
                   num_devices=N_CORES)
    for ph in phases:
        s = str(ph["b"])
        Qp, Kp = ph["Qp"], ph["Kp"]
        io = {
            "xq": nc.dram_tensor("xq" + s, [128, KCH, Qp], XDT, kind="ExternalInput"),
            "xk": nc.dram_tensor("xk" + s, [128, KCH, Kp], XDT, kind="ExternalInput"),
            "xv": nc.dram_tensor("xv" + s, [128, KCH, Kp], XDT, kind="ExternalInput"),
            "out": nc.dram_tensor("out" + s, [2, 65, Qp], F16, kind="ExternalOutput"),
        }
        ph["io"] = io

    with tile.TileContext(nc) as tc, ExitStack() as ctx:
        P = {
            "w": ctx.enter_context(tc.tile_pool(name="w", bufs=1)),
            "x": ctx.enter_context(tc.tile_pool(name="x", bufs=6)),
            "e": ctx.enter_context(tc.tile_pool(name="e", bufs=8)),
            "rows": ctx.enter_context(tc.tile_pool(name="rows", bufs=3)),
            "persist": ctx.enter_context(tc.tile_pool(name="persist", bufs=1)),
            "pp": ctx.enter_context(tc.tile_pool(name="pp", bufs=2, space="PSUM")),
            "sp": ctx.enter_context(tc.tile_pool(name="sp", bufs=4, space="PSUM")),
            "ot": ctx.enter_context(tc.tile_pool(name="ot", bufs=2, space="PSUM")),
        }
        warm = P["w"].tile([1, 1], F32, tag="actwarm", name="actwarm")
        nc.vector.memset(warm[:], 0.0)
        nc.scalar.activation(warm[:], warm[:], EXP)
        wts = {}
        for nm in ("wq", "wk", "wv"):
            wd = nc.dram_tensor(nm, [128, KCH, 128], XDT, kind="ExternalInput")
            t = P["w"].tile([128, KCH, 128], XDT, tag=nm, name=nm)
            nc.gpsimd.dma_start(t[:], wd[:])
            wts[nm] = t
        P["wts"] = wts
        for ph in phases:
            _emit_phase(nc, tc, P, ph)
    nc.compile()
    return nc


def _prep_xT(X, P):
    """[T, D] -> [128, KCH, P] with x[p, k, t] = X[t, k*128 + p]."""
    Xp = np.ascontiguousarray(X[:P].T)                 # [D, P]
    return np.ascontiguousarray(
        Xp.reshape(KCH, 128, P).transpose(1, 0, 2)).astype(XNP)  # [128, KCH, P]


def _prep_w(W, c):
    """[D, H*DH] -> per-core [128, KCH, 128] slice of heads (2c, 2c+1)."""
    Ws = W[:, c * 128:(c + 1) * 128]                   # [D, 128]
    return np.ascontiguousarray(
        Ws.reshape(KCH, 128, 128).transpose(1, 0, 2)).astype(XNP)


def kernel(Q_seq, K_seq, V_seq, Q_len, V_len, WQ, WK, WV):
    global LAST_EXEC_NS
    Q_seq = np.asarray(Q_seq, dtype=np.float32)
    K_seq = np.asarray(K_seq, dtype=np.float32)
    V_seq = np.asarray(V_seq, dtype=np.float32)
    WQ = np.asarray(WQ, dtype=np.float32)
    WK = np.asarray(WK, dtype=np.float32)
    WV = np.asarray(WV, dtype=np.float32)
    qlen = [int(np.asarray(Q_len)[b, 0]) for b in range(B)]
    vlen = [int(np.asarray(V_len)[b, 0]) for b in range(B)]

    phases = []
    for b in range(B):
        Qp = _ceil_div(qlen[b], 32) * 32   # q only needs 32-elem alignment
        if Qp == 0:
            continue  # whole batch output is zero
        if vlen[b] > 0:
            NK, scale = _ceil_div(vlen[b], 128), SCALE
        else:
            # all keys masked -> reference softmax degenerates to uniform
            # over all T keys; exp(0*S) = 1 with no v-row zeroing reproduces
            # it exactly.
            NK, scale = T // 128, 0.0
        phases.append(dict(b=b, NK=NK, Qp=Qp, Kp=NK * 128, vlen=vlen[b],
                           scale=scale, first=not phases))

    out = np.zeros((B, T, H * DH), dtype=np.float32)
    if not phases:
        return out

    nc = _build_program(phases)

    # per-phase data shared by all cores
    shared = {}
    for ph in phases:
        b, s, Qp, Kp = ph["b"], str(ph["b"]), ph["Qp"], ph["Kp"]
        xv = _prep_xT(V_seq[b], Kp)
        if 0 < vlen[b] < Kp:
            xv[:, :, vlen[b]:] = 0  # masked keys' v rows project to zero
        shared[s] = {
            "xq" + s: _prep_xT(Q_seq[b], Qp),
            "xk" + s: _prep_xT(K_seq[b], Kp),
            "xv" + s: xv,
        }

    in_maps = []
    for c in range(N_CORES):
        m = {}
        for ph in phases:
            m.update(shared[str(ph["b"])])
        m["wq"] = _prep_w(WQ, c)
        m["wk"] = _prep_w(WK, c)
        m["wv"] = _prep_w(WV, c)
        in_maps.append(m)

    trace = bool(os.environ.get("BASS_TRACE"))
    if trace:
        _ensure_ntff_hook()
    res = run_bass_kernel_spmd(nc, in_maps, list(range(N_CORES)), trace=trace)
    LAST_EXEC_NS = res.exec_time_ns

    for c in range(N_CORES):
        r = res.results[c]
        for ph in phases:
            b, s, ql = ph["b"], str(ph["b"]), qlen[ph["b"]]
            o = np.asarray(r["out" + s], dtype=np.float32)  # [2, 65, Qp]
            for h in (0, 1):
                head = 2 * c + h
                num = o[h, 0:64, :ql]                       # [64, qlen]
                den = o[h, 64, :ql]                         # [qlen]
                out[b, :ql, head * DH:(head + 1) * DH] = (num / den).T
    return out


# revision 28
# speedup vs baseline: 1.0702x; 1.0702x over previous
"""Trainium2 Bass kernel: masked multi-head attention, sharded across 8 NeuronCores.

Problem shapes (hardcoded): B=2, T=2048, D=1024, H=16 heads, dh=64.

Sharding: one SPMD program with two phases (one per batch element). In each
phase every core handles 2 of the 16 heads (core c -> heads 2c, 2c+1), so the
16 heads of each batch are spread over all 8 cores. This load-balances the
data-dependent work (Q_len/V_len trim the q/k tile counts per batch).

Device algorithm per phase, per core (all matmul operands fp16; PSUM fp32 —
the fp16 datapath runs the PE at 1 cycle/row vs fp32's 4, and halves DMA):
  - project kT [128=2*64, Kp] and qT [128, Qp] (heads stacked on partition
    halves), and v_aug [128, NK, 2, 65] (token-major with a ones-column at
    index 64 per head, so the PV matmul's psum row 64 is the softmax denom).
    Key masking is done by ZEROING the masked tokens' v_aug rows (so they
    contribute to neither numerator nor denominator) — no exp bias needed,
    which lets several key tiles share one ACT exp instruction.
  - per balanced q chunk (n = Qp/NQC), per group of G=512//n key tiles:
      S^T[kt] = kT_tile.T @ qT_chunk for each kt in group  (one PSUM bank)
      E = exp(scale*S^T)                (ONE ACT instr per group per head)
      [O^T; d] += v_aug.T @ E           (PE, K=128; psum row 64 = denom)
    with a skew-2 software pipeline (S/exp run two groups ahead of PV).
  - epilogue: single DVE copy of the raw [65, n] psum (O^T rows + denom row)
    to fp16 SBUF, DMA out. No on-device normalization.
Host divides O^T rows by the denominator row, applies the query-length mask by
writing only the first qlen rows, and transposes back to [B, T, 1024].
"""

import math
import os
from contextlib import ExitStack

import numpy as np

import concourse.bacc as bacc
import concourse.mybir as mybir
import concourse.tile as tile
from concourse.bass_utils import run_bass_kernel_spmd

F32 = mybir.dt.float32
F16 = mybir.dt.float16
F8 = mybir.dt.float8e4
DBLROW = mybir.MatmulPerfMode.DoubleRow
EXP = mybir.ActivationFunctionType.Exp
XDT = F16
XNP = np.float16
# fp8e4 E/V operands would let the PV matmul run in DoubleRow mode (2 key
# tiles per instruction at 0.5 cycles/row), but measured on this data the
# joint E+v quantization noise reaches ~3e-2 relative-to-max — over the 2e-2
# gate (and exp can overflow e4m3 to inf). Keep fp16 unless explicitly
# enabled for experiments.
USE_FP8_PV = os.environ.get("MHA_FP8_PV", "") == "1"
EDT = F8 if USE_FP8_PV else F16
# experiment: one ACT exp instruction reading a [128, 2, 512] S pair that
# spans TWO PSUM banks (suspected hardware-illegal; isolates the v2 crash)
WIDE_ACT = os.environ.get("MHA_WIDE_ACT", "") == "1"

B, T, D, H, DH = 2, 2048, 1024, 16, 64
N_CORES = 8
KCH = D // 128          # 8 contraction chunks of the model dim
SCALE = 1.0 / math.sqrt(DH)

LAST_EXEC_NS = None     # filled when BASS_TRACE=1


def _ensure_ntff_hook():
    """run_bass_kernel_spmd(trace=True) imports antenv.axon_hooks, which some
    containers lack; synthesize it (backed by libaxon_pjrt's NRT profiling)
    so tracing degrades gracefully instead of crashing."""
    import sys
    import types
    try:
        import antenv.axon_hooks  # noqa: F401
        return
    except ImportError:
        pass
    try:
        import antenv
        from trn_agent_boot.trn_boot import _ntff_profile_via_ctypes
        hook = _ntff_profile_via_ctypes("/opt/axon/libaxon_pjrt.so")
    except Exception:
        antenv = None
        hook = None
    try:
        m = types.ModuleType("antenv.axon_hooks")
        m._hook = hook
        m.set_axon_ntff_profile_hook = lambda h: setattr(m, "_hook", h)
        m.get_axon_ntff_profile_hook = lambda: m._hook
        sys.modules["antenv.axon_hooks"] = m
        if antenv is not None:
            antenv.axon_hooks = m
    except Exception:
        pass


def _ceil_div(a, b):
    return -(-a // b)


def _chunk_sizes(total, maxn):
    """Split `total` into the fewest chunks of size <= maxn, sizes balanced."""
    nch = _ceil_div(total, maxn)
    base, rem = divmod(total, nch)
    return [base + (1 if i < rem else 0) for i in range(nch)]


def _emit_phase(nc, tc, P, ph):
    """Emit one batch element's phase into the program."""
    s = str(ph["b"])
    io = ph["io"]
    NK, Qp, Kp = ph["NK"], ph["Qp"], ph["Kp"]
    vlen = ph["vlen"]
    scale = ph["scale"]
    wts = P["wts"]

    # --- k/v projections, interleaved per 512-token chunk so the PE gets
    # fresh data as soon as each chunk's DMA lands ---
    kcs = []
    vas = []
    kchunks = _chunk_sizes(Kp, 512)
    for c, n in enumerate(kchunks):
        off = sum(kchunks[:c])
        # k chunk -> kT [128(2h*64d), n]
        xt = P["x"].tile([128, KCH, n], XDT, tag="xt", name="xt")
        if ph.get("first") and c == 0:
            # per-k-slice DMAs let the first projection matmul start as
            # soon as slice 0 lands instead of after the whole chunk
            for k in range(KCH):
                nc.gpsimd.dma_start(xt[:, k, :], io["xk"][:, k, off:off + n])
        else:
            nc.gpsimd.dma_start(xt[:], io["xk"][:, :, off:off + n])
        ps = P["pp"].tile([128, n], F32, tag="pp", name="pp")
        for k in range(KCH):
            nc.tensor.matmul(ps[:], lhsT=wts["wk"][:, k, :], rhs=xt[:, k, :],
                             start=(k == 0), stop=(k == KCH - 1))
        kc = P["persist"].tile([128, n], XDT, tag="kT" + s, name="kT" + s,
                               bufs=len(kchunks))
        nc.vector.tensor_copy(kc[:], ps[:])
        kcs.append((off, n, kc))

        # v chunk -> v_aug PAIR tiles [128 tokens, 2 key tiles, 2 heads, 1+64]
        # (pairing two key tiles in one tile feeds the DoubleRow PV matmul)
        xtv = P["x"].tile([128, KCH, n], XDT, tag="xt", name="xt")
        nc.gpsimd.dma_start(xtv[:], io["xv"][:, :, off:off + n])
        for m in range(n // 128):
            kt = off // 128 + m
            j = kt % 2
            if j == 0:
                # 128-wide: cols 0-63 = v, col 64 = ones (denominator row),
                # cols 65-127 = zeros. Dual-fp8 ldweights requires each
                # matrix of the pair to be exactly 64 or 128 columns; the
                # padding rows land in unread psum rows and cost nothing
                # (matmul time only depends on the moving dimension).
                vap = P["persist"].tile([128, 2, 2, 128], EDT, tag="va" + s,
                                        name="va" + s, bufs=_ceil_div(NK, 2))
                nc.vector.memset(vap[:, :, :, 64:128], 0.0)
                vas.append(vap)
            else:
                vap = vas[-1]
            va = vap[:, j]
            if vlen > 0 and kt == NK - 1 and vlen - kt * 128 < 128:
                # masked tail keys: leave their ones-column rows zero (the
                # host already zeroed their xv columns) — they contribute to
                # neither numerator nor denominator, replacing the exp mask
                nc.vector.memset(va[0:vlen - kt * 128, :, 64:65], 1.0)
            else:
                nc.vector.memset(va[:, :, 64:65], 1.0)
            ps = P["pp"].tile([128, 128], F32, tag="pp", name="pp")
            for k in range(KCH):
                nc.tensor.matmul(ps[:], lhsT=xtv[:, k, m * 128:(m + 1) * 128],
                                 rhs=wts["wv"][:, k, :],
                                 start=(k == 0), stop=(k == KCH - 1))
            nc.vector.tensor_copy(va[:, :, 0:64],
                                  ps[:].rearrange("p (g d) -> p g d", g=2))

    def kc_slice(kt):
        for off, n, kc in kcs:
            if off <= kt * 128 < off + n:
                return kc, kt * 128 - off
        raise AssertionError

    # --- q projection + attention over balanced q chunks ---
    if WIDE_ACT:
        # bank-aligned 512-wide chunks so a kt-pair's S tiles are two whole
        # PSUM banks readable by a single exp
        qchunks = [512] * (Qp // 512) + ([Qp % 512] if Qp % 512 else [])
    else:
        qchunks = _chunk_sizes(Qp, 512)
    NQC = len(qchunks)

    def emit_qproj(c):
        n = qchunks[c]
        off = sum(qchunks[:c])
        xt = P["x"].tile([128, KCH, n], XDT, tag="xtq", name="xtq", bufs=2)
        nc.gpsimd.dma_start(xt[:], io["xq"][:, :, off:off + n])
        ps = P["pp"].tile([128, n], F32, tag="pp", name="pp")
        for k in range(KCH):
            nc.tensor.matmul(ps[:], lhsT=wts["wq"][:, k, :], rhs=xt[:, k, :],
                             start=(k == 0), stop=(k == KCH - 1))
        qc = P["persist"].tile([128, n], XDT, tag="qT" + s, name="qT" + s,
                               bufs=3)
        nc.vector.tensor_copy(qc[:], ps[:])
        return qc

    qcs = {0: emit_qproj(0)}
    for c in range(NQC):
        n = qchunks[c]
        off = sum(qchunks[:c])
        qc = qcs.pop(c)

        otd = [P["ot"].tile([128, n], F32, tag="otd", name="otd") for _ in (0, 1)]

        # key tiles are processed in pairs: each pair's E values live in one
        # [128, 2, n] tile, feeding a single DoubleRow PV matmul (fp8) or two
        # plain matmuls (fp16 fallback)
        NPAIR = _ceil_div(NK, 2)

        def emit_pair(p):
            pair = [kt for kt in (2 * p, 2 * p + 1) if kt < NK]
            es = []
            for h in (0, 1):
                e2 = P["e"].tile([128, 2, n], EDT, tag="e", name="e")
                if WIDE_ACT and n == 512 and len(pair) == 2:
                    sps = P["sp"].tile([128, 2, n], F32, tag="sps", name="sps")
                    for j, kt in enumerate(pair):
                        kc, ko = kc_slice(kt)
                        nc.tensor.matmul(
                            sps[:, j, :],
                            lhsT=kc[h * 64:(h + 1) * 64, ko:ko + 128],
                            rhs=qc[h * 64:(h + 1) * 64, :],
                            start=True, stop=True)
                    nc.scalar.activation(
                        e2[:].rearrange("p g n -> p (g n)"),
                        sps[:].rearrange("p g n -> p (g n)"), EXP, scale=scale)
                elif 2 * n <= 512 and len(pair) == 2:
                    # both S tiles fit one PSUM bank -> a single exp instr
                    sps = P["sp"].tile([128, 2, n], F32, tag="sps", name="sps")
                    for j, kt in enumerate(pair):
                        kc, ko = kc_slice(kt)
                        nc.tensor.matmul(
                            sps[:, j, :],
                            lhsT=kc[h * 64:(h + 1) * 64, ko:ko + 128],
                            rhs=qc[h * 64:(h + 1) * 64, :],
                            start=True, stop=True)
                    nc.scalar.activation(
                        e2[:].rearrange("p g n -> p (g n)"),
                        sps[:].rearrange("p g n -> p (g n)"), EXP, scale=scale)
                else:
                    for j, kt in enumerate(pair):
                        sps = P["sp"].tile([128, n], F32, tag="sps", name="sps")
                        kc, ko = kc_slice(kt)
                        nc.tensor.matmul(
                            sps[:],
                            lhsT=kc[h * 64:(h + 1) * 64, ko:ko + 128],
                            rhs=qc[h * 64:(h + 1) * 64, :],
                            start=True, stop=True)
                        nc.scalar.activation(e2[:, j, :], sps[:], EXP,
                                             scale=scale)
                es.append(e2)
            return es

        # skew-1 software pipeline over pairs (= two key tiles of lookahead):
        # S/exp of pair p+1 issue before the PV matmuls of pair p
        es_prev = emit_pair(0)
        for p in range(NPAIR):
            es_cur = es_prev
            if p + 1 < NPAIR:
                es_prev = emit_pair(p + 1)
            whole = 2 * p + 1 < NK
            for h in (0, 1):
                if whole and USE_FP8_PV:
                    nc.tensor.matmul(otd[h][:], lhsT=vas[p][:, :, h, :],
                                     rhs=es_cur[h][:],
                                     start=(p == 0), stop=(2 * p + 1 == NK - 1),
                                     perf_mode=DBLROW, skip_group_check=True)
                else:
                    for j in range(2 if whole else 1):
                        kt = 2 * p + j
                        nc.tensor.matmul(otd[h][:], lhsT=vas[p][:, j, h, :],
                                         rhs=es_cur[h][:, j, :],
                                         start=(kt == 0), stop=(kt == NK - 1),
                                         skip_group_check=True)
                        # (fp8 single / fp16 tiles are also 128 wide)
        if c + 1 < NQC:
            qcs[c + 1] = emit_qproj(c + 1)
        for h in (0, 1):
            # ship raw numerator rows + denominator row; host normalizes
            osb = P["rows"].tile([65, n], F16, tag="osb", name="osb")
            nc.vector.tensor_copy(osb[:], otd[h][0:65, :])
            nc.gpsimd.dma_start(io["out"][h, :, off:off + n], osb[:])


def _build_program(phases):
    nc = bacc.Bacc("TRN2", target_bir_lowering=False, debug=False,
                   num_devices=N_CORES)
    for ph in phases:
        s = str(ph["b"])
        Qp, Kp = ph["Qp"], ph["Kp"]
        io = {
            "xq": nc.dram_tensor("xq" + s, [128, KCH, Qp], XDT, kind="ExternalInput"),
            "xk": nc.dram_tensor("xk" + s, [128, KCH, Kp], XDT, kind="ExternalInput"),
            "xv": nc.dram_tensor("xv" + s, [128, KCH, Kp], XDT, kind="ExternalInput"),
            "out": nc.dram_tensor("out" + s, [2, 65, Qp], F16, kind="ExternalOutput"),
        }
        ph["io"] = io

    with tile.TileContext(nc) as tc, ExitStack() as ctx:
        P = {
            "w": ctx.enter_context(tc.tile_pool(name="w", bufs=1)),
            "x": ctx.enter_context(tc.tile_pool(name="x", bufs=6)),
            "e": ctx.enter_context(tc.tile_pool(name="e", bufs=8)),
            "rows": ctx.enter_context(tc.tile_pool(name="rows", bufs=3)),
            "persist": ctx.enter_context(tc.tile_pool(name="persist", bufs=1)),
            "pp": ctx.enter_context(tc.tile_pool(name="pp", bufs=2, space="PSUM")),
            "sp": ctx.enter_context(tc.tile_pool(
                name="sp", bufs=2 if WIDE_ACT else 4, space="PSUM")),
            "ot": ctx.enter_context(tc.tile_pool(name="ot", bufs=2, space="PSUM")),
        }
        warm = P["w"].tile([1, 1], F32, tag="actwarm", name="actwarm")
        nc.vector.memset(warm[:], 0.0)
        nc.scalar.activation(warm[:], warm[:], EXP)
        wts = {}
        for nm in ("wq", "wk", "wv"):
            wd = nc.dram_tensor(nm, [128, KCH, 128], XDT, kind="ExternalInput")
            t = P["w"].tile([128, KCH, 128], XDT, tag=nm, name=nm)
            nc.gpsimd.dma_start(t[:], wd[:])
            wts[nm] = t
        P["wts"] = wts
        for ph in phases:
            _emit_phase(nc, tc, P, ph)
    nc.compile()
    return nc


def _prep_xT(X, P):
    """[T, D] -> [128, KCH, P] with x[p, k, t] = X[t, k*128 + p]."""
    Xp = np.ascontiguousarray(X[:P].T)                 # [D, P]
    return np.ascontiguousarray(
        Xp.reshape(KCH, 128, P).transpose(1, 0, 2)).astype(XNP)  # [128, KCH, P]


def _prep_w(W, c):
    """[D, H*DH] -> per-core [128, KCH, 128] slice of heads (2c, 2c+1)."""
    Ws = W[:, c * 128:(c + 1) * 128]                   # [D, 128]
    return np.ascontiguousarray(
        Ws.reshape(KCH, 128, 128).transpose(1, 0, 2)).astype(XNP)


def kernel(Q_seq, K_seq, V_seq, Q_len, V_len, WQ, WK, WV):
    global LAST_EXEC_NS
    Q_seq = np.asarray(Q_seq, dtype=np.float32)
    K_seq = np.asarray(K_seq, dtype=np.float32)
    V_seq = np.asarray(V_seq, dtype=np.float32)
    WQ = np.asarray(WQ, dtype=np.float32)
    WK = np.asarray(WK, dtype=np.float32)
    WV = np.asarray(WV, dtype=np.float32)
    qlen = [int(np.asarray(Q_len)[b, 0]) for b in range(B)]
    vlen = [int(np.asarray(V_len)[b, 0]) for b in range(B)]

    phases = []
    for b in range(B):
        Qp = _ceil_div(qlen[b], 32) * 32   # q only needs 32-elem alignment
        if Qp == 0:
            continue  # whole batch output is zero
        if vlen[b] > 0:
            NK, scale = _ceil_div(vlen[b], 128), SCALE
        else:
            # all keys masked -> reference softmax degenerates to uniform
            # over all T keys; exp(0*S) = 1 with no v-row zeroing reproduces
            # it exactly.
            NK, scale = T // 128, 0.0
        phases.append(dict(b=b, NK=NK, Qp=Qp, Kp=NK * 128, vlen=vlen[b],
                           scale=scale, first=not phases))

    out = np.zeros((B, T, H * DH), dtype=np.float32)
    if not phases:
        return out

    nc = _build_program(phases)

    # per-phase data shared by all cores
    shared = {}
    for ph in phases:
        b, s, Qp, Kp = ph["b"], str(ph["b"]), ph["Qp"], ph["Kp"]
        xv = _prep_xT(V_seq[b], Kp)
        if 0 < vlen[b] < Kp:
            xv[:, :, vlen[b]:] = 0  # masked keys' v rows project to zero
        shared[s] = {
            "xq" + s: _prep_xT(Q_seq[b], Qp),
            "xk" + s: _prep_xT(K_seq[b], Kp),
            "xv" + s: xv,
        }

    in_maps = []
    for c in range(N_CORES):
        m = {}
        for ph in phases:
            m.update(shared[str(ph["b"])])
        m["wq"] = _prep_w(WQ, c)
        m["wk"] = _prep_w(WK, c)
        m["wv"] = _prep_w(WV, c)
        in_maps.append(m)

    trace = bool(os.environ.get("BASS_TRACE"))
    if trace:
        _ensure_ntff_hook()
    res = run_bass_kernel_spmd(nc, in_maps, list(range(N_CORES)), trace=trace)
    LAST_EXEC_NS = res.exec_time_ns

    for c in range(N_CORES):
        r = res.results[c]
        for ph in phases:
            b, s, ql = ph["b"], str(ph["b"]), qlen[ph["b"]]
            o = np.asarray(r["out" + s], dtype=np.float32)  # [2, 65, Qp]
            for h in (0, 1):
                head = 2 * c + h
                num = o[h, 0:64, :ql]                       # [64, qlen]
                den = o[h, 64, :ql]                         # [qlen]
                out[b, :ql, head * DH:(head + 1) * DH] = (num / den).T
    return out
